# revision 37
# baseline (speedup 1.0000x reference)
"""LocallyConnected2d Trainium2 kernel (bf16).

Problem: out[b,o,oh,ow] = sum_{c,ki,kj} x[b,c,oh+ki,ow+kj] * W[o,oh,ow,c,ki,kj] + bias[o,oh,ow]
Shapes: x[32,32,64,64], W[64,62,62,32,3,3], bias[64,62,62] -> out[32,64,62,62], fp32 in/out.

The kernel is DMA-bandwidth-bound on the weight stream (every weight is used
once per batch element, 16 FLOP/byte at fp32), so everything is shipped and
computed in bf16 (l2 rel err ~2e-3, budget 2e-2):
- Per output location: 3 accumulating PE matmuls into fp32 PSUM, K=97 each
  (chunk q = kernel row ki; features j=(kj,c) plus a ones-row at j=96 that
  carries bias on q=2).
- lhsT (stationary) = x patch columns [97,32b]: x is loaded into SBUF once as
  3 column-shifted replicas on partitions kj*32+c, so every lhsT is a direct
  AP slice (no im2col data movement). Partition 96 = memset 1.0.
- rhs (moving) = per-location weights [97,64o], streamed from HBM one output
  row at a time (2.3 MB per row, split into 3 q-chunk dma_starts so the q=0
  matmuls unblock after 1/3 of a row).
- PSUM accumulates [32b, 64o] per location; 4 locations stacked on PSUM
  partitions (PE column groups) x 8 groups = one full bank [128,512] per 32
  locations; one DVE copy per bank converts to a bf16 out strip; one
  contiguous 256KB DMA per output row.
"""

import numpy as np
from ml_dtypes import bfloat16

import concourse.bass as bass  # noqa: F401
import concourse.mybir as mybir
import concourse.tile as tile
from concourse import bacc
from concourse.bass_utils import run_bass_kernel_spmd

B, C_IN, H, W = 32, 32, 64, 64
C_OUT, OH, OW, KK = 64, 62, 62, 3
N_CORES = 8
ROWS = 8          # padded output rows per core (8*8=64 >= 62)
HALF = 31         # locations per half-row strip
XH = ROWS + 2     # input rows needed per core
KP = 97           # contraction per chunk: 96 features + ones/bias row
HZ = OW * B       # 1984 x3 free elems per input row
QZ = HALF * C_OUT  # 1984 w free elems per q chunk per strip
F32 = mybir.dt.float32
BF16 = mybir.dt.bfloat16

_NC_CACHE = {}


def _build_nc():
    nc = bacc.Bacc(
        "TRN2",
        target_bir_lowering=False,
        debug=False,
        enable_asserts=False,
        num_devices=N_CORES,
    )
    # Both big streams ship as FULL 128-partition images: a single
    # 128-partition dma_start triggers the DGE's port-local descriptor
    # swizzle and streams at ~341 GB/s vs ~180 for any narrower pattern --
    # worth the 32/128 padding overhead.
    # x: pre-shifted kj-replicas on partitions kj*32+c, ones row at 96,
    # partitions 97-127 pad.
    x_d = nc.dram_tensor("x", [128, XH, OW, B], BF16, kind="ExternalInput").ap()
    # w: [row, half, j, q, ow, o] with j=96 the bias row, 97-127 pad; one
    # contiguous 11.6KB descriptor per partition per half-row strip.
    w_d = nc.dram_tensor(
        "w", [ROWS, 2, 128, 3, HALF, C_OUT], BF16, kind="ExternalInput"
    ).ap()
    # out layout: [row, half, p=(l4,b), grp, o] - 4 locations (col groups)
    # stacked on PSUM/SBUF partitions; host unscrambles
    NG = 8  # ceil(31/4) location groups per strip
    o_d = nc.dram_tensor(
        "out", [ROWS, 2, 128, NG * C_OUT], BF16, kind="ExternalOutput"
    ).ap()

    with tile.TileContext(nc) as tc:
        with (
            tc.tile_pool(name="xpool", bufs=1) as xpool,
            tc.tile_pool(name="wpool", bufs=6) as wpool,
            tc.tile_pool(name="opool", bufs=2) as opool,
            tc.tile_pool(name="pspool", bufs=8, space="PSUM") as pspool,
        ):
            # x replicas: partition kj*32+c holds x[b,c,h,w+kj] at free
            # (h, w, b); partition 96 = 1.0 (carries the bias row).
            x3 = xpool.tile([128, XH * HZ], BF16)
            xsrc = x_d.rearrange("p h w b -> p (h w b)")
            # staged h-chunks: 0-2 covers output row 0; later chunks land
            # well ahead of their W rows (row r needs h r+2) so the
            # scheduler never defers stop-matmuls.
            for h0, h1 in ((0, 3), (3, 5), (5, 7), (7, XH)):
                nc.sync.dma_start(
                    out=x3[:, h0 * HZ : h1 * HZ], in_=xsrc[:, h0 * HZ : h1 * HZ]
                )

            for row in range(ROWS):
                for half in range(2):
                    strip = row * 2 + half
                    wt = wpool.tile([128, 3 * QZ], BF16, tag="wt")
                    # single 128-partition dma_start per strip = the ~341
                    # GB/s fast path. (Narrower dma_starts fall back to
                    # round-robin descriptor assignment with SBUF-port
                    # contention, ~180 GB/s; the scalar/Activation HWDGE
                    # ring serializes HBM->SBUF loads on one engine. Not
                    # split by q chunk: staggered q arrival makes the
                    # scheduler defer q=2 stop-matmuls, piling up open PSUM
                    # groups.)
                    wsrc = w_d[row, half].rearrange("p q l o -> p (q l o)")
                    nc.gpsimd.dma_start(out=wt, in_=wsrc)
                    ot = opool.tile([128, NG * C_OUT], BF16, tag="ot")
                    otv = ot.rearrange("p (g o) -> p g o", g=NG, o=C_OUT)
                    for g in range(NG):
                        gn = min(4, HALF - g * 4)  # 4,...,4,3
                        # 4 locations packed into PE col groups: out slice
                        # base partition 32*l selects the col group, so the
                        # 4 locations' matmuls can overlap in the array
                        ps = pspool.tile([128, C_OUT], F32, tag="ps")
                        for li in range(4):
                            # pad slot in the last group duplicates the
                            # prior location (keeps PSUM fully written; host
                            # drops it). li outer / q inner: each location's
                            # start..stop accumulation nests fully before
                            # the next starts (the sim's zero-region
                            # tracking aliases partition-sliced PSUM
                            # offsets, so interleaved starts trip it).
                            eff = min(li, gn - 1)
                            lo = g * 4 + eff
                            ow = half * HALF + lo
                            for q in range(3):
                                nc.tensor.matmul(
                                    ps[32 * li : 32 * li + 32, :],
                                    x3[0:KP, (row + q) * HZ + ow * B : (row + q) * HZ + ow * B + B],
                                    wt[0:KP, q * QZ + lo * C_OUT : q * QZ + lo * C_OUT + C_OUT],
                                    start=(q == 0),
                                    stop=(q == 2),
                                    tile_position=(0, 32 * li),
                                )
                        nc.vector.tensor_copy(out=otv[:, g, :], in_=ps)
                    # scalar HWDGE ring spreads SBUF->HBM stores fine and
                    # keeps them off the gpsimd FIFO so W prefetch is never
                    # head-of-line blocked. Last strips go via gpsimd (idle
                    # by then) to shrink the tail.
                    oeng = nc.gpsimd if strip >= 14 else nc.scalar
                    oeng.dma_start(out=o_d[row, half], in_=ot)

    nc.compile()
    return nc


def get_nc():
    if "nc" not in _NC_CACHE:
        _NC_CACHE["nc"] = _build_nc()
    return _NC_CACHE["nc"]


def prep_inputs(x, weight, bias):
    """Host-side shard + layout prep. Returns per-core in_maps."""
    x = np.asarray(x, dtype=np.float32)
    weight = np.asarray(weight, dtype=np.float32)
    bias = np.asarray(bias, dtype=np.float32)

    # w_prep[oh, j=kj*32+c, q=ki, ow, o]; j=96 row: 0 for q<2, bias for q=2;
    # j=97..127 pad (128-partition DMA fast path); then split ow into
    # half-row strips: [row, half, j, q, l, o]
    wp = np.zeros((N_CORES * ROWS, 128, 3, OW, C_OUT), np.float32)
    wp[:OH, :96] = weight.transpose(1, 5, 3, 4, 2, 0).reshape(OH, 96, 3, OW, C_OUT)
    wp[:OH, 96, 2] = bias.transpose(1, 2, 0)
    wp = wp.astype(bfloat16)
    wp = np.ascontiguousarray(
        wp.reshape(N_CORES * ROWS, 128, 3, 2, HALF, C_OUT).transpose(0, 3, 1, 2, 4, 5)
    )

    xp = np.zeros((B, C_IN, N_CORES * ROWS + 2, W), np.float32)
    xp[:, :, :H] = x
    xt = xp.transpose(1, 2, 3, 0).astype(bfloat16)  # [c, h, w, b]

    in_maps = []
    for c in range(N_CORES):
        r0 = c * ROWS
        xc = xt[:, r0 : r0 + XH]  # [c, 10, 64, b]
        xsh = np.zeros((128, XH, OW, B), bfloat16)
        for kj in range(KK):
            xsh[kj * 32 : kj * 32 + 32] = xc[:, :, kj : kj + OW, :]
        xsh[96] = 1.0
        in_maps.append(
            {
                "x": xsh,
                "w": np.ascontiguousarray(wp[r0 : r0 + ROWS]),
            }
        )
    return in_maps


def gather_output(results):
    """results: list of per-core out dicts -> full [B, C_OUT, OH, OW]."""
    out = np.empty((B, C_OUT, OH, OW), np.float32)
    for c in range(N_CORES):
        oc = np.asarray(results[c]["out"], dtype=np.float32)  # [ROWS, 2, 128, 8*C_OUT]
        v = oc.reshape(ROWS, 2, 4, B, 8, C_OUT)  # [r, half, l, b, g, o]
        # ow = half*31 + 4*g + l  (only 4*g+l < 31 valid)
        arr = v.transpose(3, 5, 0, 1, 4, 2).reshape(B, C_OUT, ROWS, 2, 32)
        arr = arr[:, :, :, :, :HALF].reshape(B, C_OUT, ROWS, OW)
        r0 = c * ROWS
        rows = min(ROWS, OH - r0)
        out[:, :, r0 : r0 + rows, :] = arr[:, :, :rows, :]
    return out


def run(inputs, **kw):
    nc = get_nc()
    in_maps = prep_inputs(inputs["x"], inputs["weight"], inputs["bias"])
    res = run_bass_kernel_spmd(nc, in_maps, core_ids=list(range(N_CORES)), **kw)
    return gather_output(res.results), res


def kernel(x, weight, bias):
    out, _ = run({"x": x, "weight": weight, "bias": bias})
    return out


# revision 38
# speedup vs baseline: 1.0415x; 1.0415x over previous
"""LocallyConnected2d Trainium2 kernel (bf16).

Problem: out[b,o,oh,ow] = sum_{c,ki,kj} x[b,c,oh+ki,ow+kj] * W[o,oh,ow,c,ki,kj] + bias[o,oh,ow]
Shapes: x[32,32,64,64], W[64,62,62,32,3,3], bias[64,62,62] -> out[32,64,62,62], fp32 in/out.

The kernel is DMA-bandwidth-bound on the weight stream (every weight is used
once per batch element, 16 FLOP/byte at fp32), so everything is shipped and
computed in bf16 (l2 rel err ~2e-3, budget 2e-2):
- Per output location: 3 accumulating PE matmuls into fp32 PSUM, K=97 each
  (chunk q = kernel row ki; features j=(kj,c) plus a ones-row at j=96 that
  carries bias on q=2).
- lhsT (stationary) = x patch columns [97,32b]: x is loaded into SBUF once as
  3 column-shifted replicas on partitions kj*32+c, so every lhsT is a direct
  AP slice (no im2col data movement). Partition 96 = memset 1.0.
- rhs (moving) = per-location weights [97,64o], streamed from HBM one output
  row at a time (2.3 MB per row, split into 3 q-chunk dma_starts so the q=0
  matmuls unblock after 1/3 of a row).
- PSUM accumulates [32b, 64o] per location; 4 locations stacked on PSUM
  partitions (PE column groups) x 8 groups = one full bank [128,512] per 32
  locations; one DVE copy per bank converts to a bf16 out strip; one
  contiguous 256KB DMA per output row.
"""

import numpy as np
from ml_dtypes import bfloat16

import concourse.bass as bass  # noqa: F401
import concourse.mybir as mybir
import concourse.tile as tile
from concourse import bacc
from concourse.bass_utils import run_bass_kernel_spmd

B, C_IN, H, W = 32, 32, 64, 64
C_OUT, OH, OW, KK = 64, 62, 62, 3
N_CORES = 8
ROWS = 8          # padded output rows per core (8*8=64 >= 62)
HALF = 31         # locations per half-row strip
XH = ROWS + 2     # input rows needed per core
KP = 97           # contraction per chunk: 96 features + ones/bias row
HZ = OW * B       # 1984 x3 free elems per input row
QZ = HALF * C_OUT  # 1984 w free elems per q chunk per strip
F32 = mybir.dt.float32
BF16 = mybir.dt.bfloat16

_NC_CACHE = {}


def _build_nc():
    nc = bacc.Bacc(
        "TRN2",
        target_bir_lowering=False,
        debug=False,
        enable_asserts=False,
        num_devices=N_CORES,
    )
    # Both big streams ship as FULL 128-partition images: a single
    # 128-partition dma_start triggers the DGE's port-local descriptor
    # swizzle and streams at ~341 GB/s vs ~180 for any narrower pattern --
    # worth the 32/128 padding overhead.
    # x: pre-shifted kj-replicas on partitions kj*32+c, ones row at 96,
    # partitions 97-127 pad.
    x_d = nc.dram_tensor("x", [128, XH, OW, B], BF16, kind="ExternalInput").ap()
    # w: [row, half, j, q, ow, o] with j=96 the bias row, 97-127 pad; one
    # contiguous 11.6KB descriptor per partition per half-row strip.
    w_d = nc.dram_tensor(
        "w", [ROWS, 2, 128, 3, HALF, C_OUT], BF16, kind="ExternalInput"
    ).ap()
    # out layout: [row, half, p=(l4,b), grp, o] - 4 locations (col groups)
    # stacked on PSUM/SBUF partitions; host unscrambles
    NG = 8  # ceil(31/4) location groups per strip
    o_d = nc.dram_tensor(
        "out", [ROWS, 2, 128, NG * C_OUT], BF16, kind="ExternalOutput"
    ).ap()

    with tile.TileContext(nc) as tc:
        with (
            tc.tile_pool(name="xpool", bufs=1) as xpool,
            tc.tile_pool(name="wpool", bufs=8) as wpool,
            tc.tile_pool(name="opool", bufs=2) as opool,
            tc.tile_pool(name="pspool", bufs=8, space="PSUM") as pspool,
        ):
            # x replicas: partition kj*32+c holds x[b,c,h,w+kj] at free
            # (h, w, b); partition 96 = 1.0 (carries the bias row).
            x3 = xpool.tile([128, XH * HZ], BF16)
            xsrc = x_d.rearrange("p h w b -> p (h w b)")
            # staged h-chunks: 0-2 covers output row 0; later chunks land
            # well ahead of their W rows (row r needs h r+2) so the
            # scheduler never defers stop-matmuls.
            for h0, h1 in ((0, 3), (3, 5), (5, 7), (7, XH)):
                nc.sync.dma_start(
                    out=x3[:, h0 * HZ : h1 * HZ], in_=xsrc[:, h0 * HZ : h1 * HZ]
                )

            for row in range(ROWS):
                for half in range(2):
                    strip = row * 2 + half
                    wt = wpool.tile([128, 3 * QZ], BF16, tag="wt")
                    # single 128-partition dma_start per strip = the ~341
                    # GB/s fast path. (Narrower dma_starts fall back to
                    # round-robin descriptor assignment with SBUF-port
                    # contention, ~180 GB/s; the scalar/Activation HWDGE
                    # ring serializes HBM->SBUF loads on one engine. Not
                    # split by q chunk: staggered q arrival makes the
                    # scheduler defer q=2 stop-matmuls, piling up open PSUM
                    # groups.)
                    wsrc = w_d[row, half].rearrange("p q l o -> p (q l o)")
                    nc.gpsimd.dma_start(out=wt, in_=wsrc)
                    ot = opool.tile([128, NG * C_OUT], BF16, tag="ot")
                    otv = ot.rearrange("p (g o) -> p g o", g=NG, o=C_OUT)
                    for g in range(NG):
                        gn = min(4, HALF - g * 4)  # 4,...,4,3
                        # 4 locations packed into PE col groups: out slice
                        # base partition 32*l selects the col group, so the
                        # 4 locations' matmuls can overlap in the array
                        ps = pspool.tile([128, C_OUT], F32, tag="ps")
                        for li in range(4):
                            # pad slot in the last group duplicates the
                            # prior location (keeps PSUM fully written; host
                            # drops it). li outer / q inner: each location's
                            # start..stop accumulation nests fully before
                            # the next starts (the sim's zero-region
                            # tracking aliases partition-sliced PSUM
                            # offsets, so interleaved starts trip it).
                            eff = min(li, gn - 1)
                            lo = g * 4 + eff
                            ow = half * HALF + lo
                            for q in range(3):
                                nc.tensor.matmul(
                                    ps[32 * li : 32 * li + 32, :],
                                    x3[0:KP, (row + q) * HZ + ow * B : (row + q) * HZ + ow * B + B],
                                    wt[0:KP, q * QZ + lo * C_OUT : q * QZ + lo * C_OUT + C_OUT],
                                    start=(q == 0),
                                    stop=(q == 2),
                                    tile_position=(0, 32 * li),
                                )
                        nc.vector.tensor_copy(out=otv[:, g, :], in_=ps)
                    # scalar HWDGE ring spreads SBUF->HBM stores fine and
                    # keeps them off the gpsimd FIFO so W prefetch is never
                    # head-of-line blocked. Last strips go via gpsimd (idle
                    # by then) to shrink the tail.
                    oeng = nc.gpsimd if strip >= 14 else nc.scalar
                    oeng.dma_start(out=o_d[row, half], in_=ot)

    nc.compile()
    return nc


def get_nc():
    if "nc" not in _NC_CACHE:
        _NC_CACHE["nc"] = _build_nc()
    return _NC_CACHE["nc"]


def prep_inputs(x, weight, bias):
    """Host-side shard + layout prep. Returns per-core in_maps."""
    x = np.asarray(x, dtype=np.float32)
    weight = np.asarray(weight, dtype=np.float32)
    bias = np.asarray(bias, dtype=np.float32)

    # w_prep[oh, j=kj*32+c, q=ki, ow, o]; j=96 row: 0 for q<2, bias for q=2;
    # j=97..127 pad (128-partition DMA fast path); then split ow into
    # half-row strips: [row, half, j, q, l, o]
    wp = np.zeros((N_CORES * ROWS, 128, 3, OW, C_OUT), np.float32)
    wp[:OH, :96] = weight.transpose(1, 5, 3, 4, 2, 0).reshape(OH, 96, 3, OW, C_OUT)
    wp[:OH, 96, 2] = bias.transpose(1, 2, 0)
    wp = wp.astype(bfloat16)
    wp = np.ascontiguousarray(
        wp.reshape(N_CORES * ROWS, 128, 3, 2, HALF, C_OUT).transpose(0, 3, 1, 2, 4, 5)
    )

    xp = np.zeros((B, C_IN, N_CORES * ROWS + 2, W), np.float32)
    xp[:, :, :H] = x
    xt = xp.transpose(1, 2, 3, 0).astype(bfloat16)  # [c, h, w, b]

    in_maps = []
    for c in range(N_CORES):
        r0 = c * ROWS
        xc = xt[:, r0 : r0 + XH]  # [c, 10, 64, b]
        xsh = np.zeros((128, XH, OW, B), bfloat16)
        for kj in range(KK):
            xsh[kj * 32 : kj * 32 + 32] = xc[:, :, kj : kj + OW, :]
        xsh[96] = 1.0
        in_maps.append(
            {
                "x": xsh,
                "w": np.ascontiguousarray(wp[r0 : r0 + ROWS]),
            }
        )
    return in_maps


def gather_output(results):
    """results: list of per-core out dicts -> full [B, C_OUT, OH, OW]."""
    out = np.empty((B, C_OUT, OH, OW), np.float32)
    for c in range(N_CORES):
        oc = np.asarray(results[c]["out"], dtype=np.float32)  # [ROWS, 2, 128, 8*C_OUT]
        v = oc.reshape(ROWS, 2, 4, B, 8, C_OUT)  # [r, half, l, b, g, o]
        # ow = half*31 + 4*g + l  (only 4*g+l < 31 valid)
        arr = v.transpose(3, 5, 0, 1, 4, 2).reshape(B, C_OUT, ROWS, 2, 32)
        arr = arr[:, :, :, :, :HALF].reshape(B, C_OUT, ROWS, OW)
        r0 = c * ROWS
        rows = min(ROWS, OH - r0)
        out[:, :, r0 : r0 + rows, :] = arr[:, :, :rows, :]
    return out


def run(inputs, **kw):
    nc = get_nc()
    in_maps = prep_inputs(inputs["x"], inputs["weight"], inputs["bias"])
    res = run_bass_kernel_spmd(nc, in_maps, core_ids=list(range(N_CORES)), **kw)
    return gather_output(res.results), res


def kernel(x, weight, bias):
    out, _ = run({"x": x, "weight": weight, "bias": bias})
    return out
